# revision 23
# baseline (speedup 1.0000x reference)
"""Multi-head self-attention (RoPE + diagonal mask) TRN2 Bass kernel, 8-core SPMD.

Sharding: core = batch*2 + head_half. Each core computes, for its batch and its
8 heads: QKV projection (fp16 matmuls, f32 PSUM), RoPE, transposed scores
S^T = K @ Q^T (no P-transpose needed), exp (no max-subtraction - scores are
bounded), diagonal mask as post-exp zeroing, P^T @ V with a ones-column to get
the softmax denominators, normalization, and the output projection restricted
to its heads' rows of Wproj. The two cores sharing a batch return partial
projection outputs which the host sums (tensor-parallel reduce).

Engine budget per core: ScalarE runs only the 256 exp ops (the softmax
numerator), everything elementwise/evac is on VectorE, and the QKV/projection
matmuls are interleaved with the ACT-bound attention inner loop through a
dedicated 2-bank PSUM pool so the PE stays fed.
"""
import sys

sys.path.insert(0, "/opt/trn_rl_repo")

import numpy as np

import concourse.bass as bass
import concourse.mybir as mybir
import concourse.tile as tile
from concourse import bacc
from concourse.bass_utils import run_bass_kernel_spmd

FP16 = mybir.dt.float16
F32 = mybir.dt.float32

B = 4
S = 2048
DM = 1024
NH = 16
HD = 64
H_CORE = 8          # heads per core
N_CORES = 8
KT = DM // 128      # 8 k-tiles over the model dim
SC = S // 128       # 16 seq chunks of 128
SCALE = HD ** -0.5

SWAP_MASK = []
for _i in range(16):
    SWAP_MASK += [2 * _i + 1, 2 * _i]

_CACHE = {}


def _build_nc():
    nc = bacc.Bacc("TRN2", target_bir_lowering=False, debug=False, num_devices=N_CORES)

    xT_d = nc.dram_tensor("xT", [DM, S], FP16, kind="ExternalInput").ap()
    wq_d = nc.dram_tensor("wq", [DM, 512], FP16, kind="ExternalInput").ap()
    wk_d = nc.dram_tensor("wk", [DM, 512], FP16, kind="ExternalInput").ap()
    wv_d = nc.dram_tensor("wv", [DM, 512], FP16, kind="ExternalInput").ap()
    wp_d = nc.dram_tensor("wp", [512, DM], FP16, kind="ExternalInput").ap()
    cos_d = nc.dram_tensor("cosb", [128, S], FP16, kind="ExternalInput").ap()
    sin_d = nc.dram_tensor("sinb", [128, S], FP16, kind="ExternalInput").ap()
    msk_d = nc.dram_tensor("dmask", [128, 128], FP16, kind="ExternalInput").ap()
    out_d = [
        nc.dram_tensor(f"out{t}", [S, DM], F32, kind="ExternalOutput").ap()
        for t in range(4)
    ]

    zd = nc.dram_tensor("zd", [H_CORE, S], F32).ap()
    rzd = nc.dram_tensor("rzd", [H_CORE, S], F32).ap()

    Exp = mybir.ActivationFunctionType.Exp

    with tile.TileContext(nc) as tc:
        with (
            tc.tile_pool(name="consts", bufs=1) as consts,
            tc.tile_pool(name="phb", bufs=1) as phb,
            tc.tile_pool(name="rope", bufs=2) as ropep,
            tc.tile_pool(name="pt", bufs=4) as ptp,
            tc.tile_pool(name="yaug", bufs=2) as yaugp,
            tc.tile_pool(name="rzb", bufs=2) as rzbp,
            tc.tile_pool(name="zc", bufs=4) as zcp,
            tc.tile_pool(name="outsb", bufs=3) as outp,
            tc.tile_pool(name="sps", bufs=2, space="PSUM") as spsp,
            tc.tile_pool(name="pvps", bufs=1, space="PSUM") as pvpsp,
            tc.tile_pool(name="aux", bufs=2, space="PSUM") as auxp,
        ):
            # ---- persistent tiles ----
            cos_sb = consts.tile([128, S], FP16)
            sin_sb = consts.tile([128, S], FP16)
            msk_sb = consts.tile([128, 128], FP16)
            wp_sb = consts.tile([128, 4, DM], FP16)

            kT = [consts.tile([128, S], FP16, name=f"kT{t}", tag=f"kT{t}") for t in range(4)]
            qT = [consts.tile([128, S], FP16, name=f"qT{t}", tag=f"qT{t}") for t in range(4)]
            yn = [consts.tile([128, S], FP16, name=f"yn{t}", tag=f"yn{t}") for t in range(4)]
            v_sb = consts.tile([128, SC, H_CORE, HD + 1], FP16)
            nc.vector.memset(v_sb[:, :, :, HD:HD + 1], 1.0)

            # ---- inputs for the projections (released with phb) ----
            xT_sb = phb.tile([128, KT, S], FP16)
            wq_sb = phb.tile([128, KT, 512], FP16)
            wk_sb = phb.tile([128, KT, 512], FP16)
            wv_sb = phb.tile([128, KT, 512], FP16)
            _dma_engines = [nc.sync, nc.gpsimd, nc.scalar]
            for kt in range(KT):
                sl = slice(kt * 128, (kt + 1) * 128)
                _dma_engines[kt % 3].dma_start(out=xT_sb[:, kt, :], in_=xT_d[sl, :])
                _dma_engines[(kt + 1) % 3].dma_start(out=wq_sb[:, kt, :], in_=wq_d[sl, :])
                _dma_engines[(kt + 2) % 3].dma_start(out=wk_sb[:, kt, :], in_=wk_d[sl, :])
                _dma_engines[(kt + 3) % 3].dma_start(out=wv_sb[:, kt, :], in_=wv_d[sl, :])
            nc.sync.dma_start(out=cos_sb, in_=cos_d)
            nc.gpsimd.dma_start(out=sin_sb, in_=sin_d)
            nc.sync.dma_start(out=msk_sb, in_=msk_d)
            for t4 in range(4):
                nc.gpsimd.dma_start(out=wp_sb[:, t4, :], in_=wp_d[t4 * 128:(t4 + 1) * 128, :])

            def accum512(dst_view, lhsT_of_kt, rhs_of_kt, name, src_rearrange=None):
                """8-step k-accumulation into a [128, 512] aux psum, evac'd to dst."""
                ps = auxp.tile([128, 512], F32, tag="aux", name=name)
                for kt in range(KT):
                    nc.tensor.matmul(
                        ps[:], lhsT_of_kt(kt), rhs_of_kt(kt),
                        start=(kt == 0), stop=(kt == KT - 1),
                    )
                src = ps[:] if src_rearrange is None else ps[:].rearrange(*src_rearrange, d=HD)
                nc.vector.tensor_copy(dst_view, src)

            def emit_v(sc):
                accum512(
                    v_sb[:, sc, :, 0:HD],
                    lambda kt, sc=sc: xT_sb[:, kt, sc * 128:(sc + 1) * 128],
                    lambda kt: wv_sb[:, kt, :],
                    name=f"vps{sc}",
                    src_rearrange=("p (h d) -> p h d",),
                )

            rope_raw = {}

            def emit_kq_quarter(t, which, qc):
                w_sb = wk_sb if which == 0 else wq_sb
                if qc == 0:
                    rope_raw[(t, which)] = ropep.tile(
                        [128, S], FP16, tag="raw", bufs=2, name=f"raw{t}_{which}")
                raw = rope_raw[(t, which)]
                accum512(
                    raw[:, qc * 512:(qc + 1) * 512],
                    lambda kt, t=t, w_sb=w_sb: w_sb[:, kt, t * 128:(t + 1) * 128],
                    lambda kt, qc=qc: xT_sb[:, kt, qc * 512:(qc + 1) * 512],
                    name=f"kq{t}_{which}_{qc}",
                )

            def emit_rope(t, which):
                raw = rope_raw.pop((t, which))
                dst = kT if which == 0 else qT
                sw = ropep.tile([128, S], FP16, tag="sw", name=f"sw{t}_{which}")
                nc.vector.stream_shuffle(sw[:], raw[:], SWAP_MASK)
                nc.vector.tensor_mul(raw[:], raw[:], cos_sb[:])
                nc.vector.tensor_mul(sw[:], sw[:], sin_sb[:])
                nc.vector.tensor_add(dst[t][:], raw[:], sw[:])

            def emit_proj(t, sc, evac_act=False):
                osb = outp.tile([128, DM], F32, tag="osb", name=f"osb{t}_{sc}")
                for nn in range(2):
                    pp = auxp.tile([128, 512], F32, tag="aux", name=f"pp{t}_{sc}_{nn}")
                    nc.tensor.matmul(
                        pp[:],
                        yn[t][:, sc * 128:(sc + 1) * 128],
                        wp_sb[:, t, nn * 512:(nn + 1) * 512],
                        start=True,
                        stop=True,
                    )
                    if evac_act and nn == 1:
                        nc.scalar.copy(osb[:, nn * 512:(nn + 1) * 512], pp[:])
                    else:
                        nc.vector.tensor_copy(osb[:, nn * 512:(nn + 1) * 512], pp[:])
                _dma_engines[sc % 2].dma_start(
                    out=out_d[t][sc * 128:(sc + 1) * 128, :], in_=osb[:])

            fillers = []

            def drip(n=1):
                for _ in range(n):
                    if fillers:
                        fillers.pop(0)()

            # ---- prologue: K/Q/RoPE for tile 0 first (gates the first exp),
            # then only V chunk 0; V(1..15) drip just-in-time as fillers ----
            for which in (0, 1):
                for qc in range(4):
                    emit_kq_quarter(0, which, qc)
                emit_rope(0, which)
            emit_v(0)

            # ---- per head: attention with interleaved filler work ----
            for t in range(4):
                for par in range(2):
                    h = 2 * t + par
                    if h == 0:
                        fillers += [lambda sc=sc: emit_v(sc) for sc in range(1, SC)]
                    elif par == 1 and t < 3:
                        for which in (0, 1):
                            fillers += [
                                lambda w=which, qc=qc, t=t: emit_kq_quarter(t + 1, w, qc)
                                for qc in range(4)
                            ]
                            fillers.append(lambda w=which, t=t: emit_rope(t + 1, w))
                    elif par == 0 and t >= 1:
                        fillers += [lambda sc=sc, t=t: emit_proj(t - 1, sc) for sc in range(SC)]

                    rows = slice(64 * par, 64 * par + 64)
                    ya = yaugp.tile([65, S], F32, tag="ya", name=f"ya{h}")
                    for qh in range(2):
                        pv = pvpsp.tile([65, 1024], F32, tag="pv", name=f"pv{h}_{qh}")
                        for kc in range(SC):
                            pt = ptp.tile([128, 1024], FP16, tag="pt", name=f"pt{h}_{qh}_{kc}")
                            sps = spsp.tile([128, 1024], F32, tag="s", name=f"s{h}_{qh}_{kc}")
                            for qg in range(2):
                                q0 = qh * 1024 + qg * 512
                                nc.tensor.matmul(
                                    sps[:, qg * 512:(qg + 1) * 512],
                                    kT[t][rows, kc * 128:(kc + 1) * 128],
                                    qT[t][rows, q0:q0 + 512],
                                    start=True,
                                    stop=True,
                                )
                            nc.scalar.activation(pt[:], sps[:], Exp, scale=SCALE)
                            if kc // 8 == qh:
                                c0 = kc * 128 - qh * 1024
                                nc.gpsimd.tensor_mul(
                                    pt[:, c0:c0 + 128], pt[:, c0:c0 + 128], msk_sb[:])
                            for qg in range(2):
                                nc.tensor.matmul(
                                    pv[:, qg * 512:(qg + 1) * 512],
                                    v_sb[:, kc, h, :],
                                    pt[:, qg * 512:(qg + 1) * 512],
                                    start=(kc == 0),
                                    stop=(kc == SC - 1),
                                )
                            drip(1)
                        nc.vector.tensor_copy(ya[:, qh * 1024:(qh + 1) * 1024], pv[:])
                        # softmax denominators -> normalize (per q-half, pipelined)
                        q0 = qh * 1024
                        nc.sync.dma_start(
                            out=zd[h:h + 1, q0:q0 + 1024], in_=ya[64:65, q0:q0 + 1024])
                        zc = zcp.tile([8, 128], F32, tag="zc", name=f"zc{h}_{qh}")
                        nc.sync.dma_start(
                            out=zc,
                            in_=zd[h, q0:q0 + 1024].rearrange("(a b) -> a b", a=8))
                        rz = zcp.tile([8, 128], F32, tag="rz", name=f"rz{h}_{qh}")
                        nc.vector.reciprocal(rz[:], zc[:])
                        nc.sync.dma_start(
                            out=rzd[h, q0:q0 + 1024].rearrange("(a b) -> a b", a=8),
                            in_=rz[:])
                        rzb = rzbp.tile([64, 1024], F32, tag="rzb", name=f"rzb{h}_{qh}")
                        nc.sync.dma_start(
                            out=rzb,
                            in_=bass.AP(tensor=rzd.tensor, offset=h * S + q0,
                                        ap=[[0, 64], [1, 1024]]),
                        )
                        nc.vector.tensor_mul(
                            yn[t][rows, q0:q0 + 1024], ya[0:64, q0:q0 + 1024], rzb[:])

            drip(len(fillers))
            for sc in range(SC):
                emit_proj(3, sc, evac_act=True)

    nc.compile()
    return nc


def _host_tables():
    theta = 1.0 / (10000.0 ** (np.arange(0, HD, 2, dtype=np.float32) / HD))
    ang = np.arange(S, dtype=np.float32)[:, None] * theta[None, :]  # [S, 32]
    cos = np.repeat(np.cos(ang).T, 2, axis=0)  # [64, S]
    sin_ = np.empty((HD, S), np.float32)
    sin_[0::2] = -np.sin(ang).T
    sin_[1::2] = np.sin(ang).T
    cosb = np.concatenate([cos, cos], axis=0).astype(np.float16)  # [128, S]
    sinb = np.concatenate([sin_, sin_], axis=0).astype(np.float16)
    dmask = (1.0 - np.eye(128, dtype=np.float32)).astype(np.float16)
    return cosb, sinb, dmask


def _in_maps(x, Wqkv, Wproj):
    cosb, sinb, dmask = _host_tables()
    maps = []
    for core in range(N_CORES):
        b, hh = divmod(core, 2)
        c0 = hh * 512
        maps.append(
            {
                "xT": np.ascontiguousarray(x[b].T).astype(np.float16),
                "wq": np.ascontiguousarray(Wqkv[:, c0:c0 + 512]).astype(np.float16),
                "wk": np.ascontiguousarray(Wqkv[:, DM + c0:DM + c0 + 512]).astype(np.float16),
                "wv": np.ascontiguousarray(Wqkv[:, 2 * DM + c0:2 * DM + c0 + 512]).astype(np.float16),
                "wp": np.ascontiguousarray(Wproj[c0:c0 + 512, :]).astype(np.float16),
                "cosb": cosb,
                "sinb": sinb,
                "dmask": dmask,
            }
        )
    return maps


def kernel(x, Wqkv, Wproj):
    if "nc" not in _CACHE:
        _CACHE["nc"] = _build_nc()
    nc = _CACHE["nc"]

    x = np.asarray(x)
    Wqkv = np.asarray(Wqkv)
    Wproj = np.asarray(Wproj)

    res = run_bass_kernel_spmd(nc, _in_maps(x, Wqkv, Wproj), core_ids=list(range(N_CORES)))
    out = np.empty((B, S, DM), np.float32)
    for b in range(B):
        acc = None
        for core in (2 * b, 2 * b + 1):
            for t in range(4):
                part = res.results[core][f"out{t}"]
                acc = part if acc is None else acc + part
        out[b] = acc
    return out
